# revision 16
# baseline (speedup 1.0000x reference)
"""CountVectorizer (histogram + linear projection) Trainium2 kernel.

Math: proj[b, :] = sum_l W[:, token_ids[b, l]] + bias  (counts @ W.T + b),
i.e. an embedding gather-sum over tokens -- the [B, V] histogram is never
materialized.

Sharding: data-parallel over documents, 128 docs per core x 8 cores.
W is replicated to every core as a pre-transposed bf16 table WT [V, D].

Per-core device program:
  - dma_gather (SWDGE indirect DMA) fetches one WT row (1536 B) per token,
    l-major, so gathered block l = [128 docs, 768] for token position l.
  - PE accumulates all 200 blocks into PSUM fp32 via identity-lhsT matmuls
    (out[b, :] += gathered[b, :]).
  - DVE adds bias while copying PSUM->SBUF, then reduce_sum + is_equal
    produce the padding mask.
"""

import numpy as np
import ml_dtypes

B, L, V, D = 1024, 200, 32000, 768
NCORES = 8
DPC = B // NCORES  # 128 docs per core
NBLK = 8           # gathered 128-row blocks per dma_gather call (1024 idxs;
                   # >=1536 idxs/call crashes the SWDGE ucode on this runtime)
NCHUNK = L // NBLK

_CACHE = {}


def _build_module():
    import concourse.mybir as mybir
    import concourse.tile as tile
    from concourse import bacc

    # 4 SWDGE queues: descriptor generation for consecutive gather chunks runs
    # on different Q7 cores concurrently (~8.6us/1024-idx chunk each, but 4 in
    # flight) instead of serializing on one queue.
    nc = bacc.Bacc(
        "TRN2", debug=False, num_swdge_queues=4, dynamic_dma_scratch_size=65536
    )

    wt = nc.dram_tensor("wt", [V, D], mybir.dt.bfloat16, kind="ExternalInput")
    idx = nc.dram_tensor("idx", [128, L * 8], mybir.dt.int16, kind="ExternalInput")
    ident = nc.dram_tensor("ident", [128, 128], mybir.dt.bfloat16, kind="ExternalInput")
    bias = nc.dram_tensor("bias", [128, D], mybir.dt.float32, kind="ExternalInput")
    proj = nc.dram_tensor("proj", [DPC, D], mybir.dt.float32, kind="ExternalOutput")
    mask = nc.dram_tensor("mask", [DPC, 1], mybir.dt.float32, kind="ExternalOutput")

    with tile.TileContext(nc) as tc:
        with (
            tc.tile_pool(name="const", bufs=1) as cpool,
            tc.tile_pool(name="gath", bufs=10) as gpool,
            tc.tile_pool(name="outp", bufs=1) as opool,
            tc.tile_pool(name="ps", bufs=1, space="PSUM") as pspool,
        ):
            # Dependency-free dummy gather: forces the GPSIMD ucode library
            # load at t~=0 (otherwise it serializes behind the idx DMA wait of
            # the first real gather, costing ~10us of startup).
            warm_idx = cpool.tile([128, 8], mybir.dt.int16)
            nc.gpsimd.memset(warm_idx[:], 0)
            warm_g = cpool.tile([128, 1, D], mybir.dt.bfloat16)
            nc.gpsimd.dma_gather(
                warm_g[:],
                wt[:, :],
                warm_idx[:],
                num_idxs=128,
                num_idxs_reg=128,
                elem_size=D,
                queue_num=3,
            )

            ident_sb = cpool.tile([128, 128], mybir.dt.bfloat16)
            nc.sync.dma_start(ident_sb[:], ident[:])
            idx_sb = cpool.tile([128, L * 8], mybir.dt.int16)
            nc.sync.dma_start(idx_sb[:], idx[:])
            bias_sb = cpool.tile([128, D], mybir.dt.float32)
            nc.sync.dma_start(bias_sb[:], bias[:])

            ps0 = pspool.tile([128, 512], mybir.dt.float32)
            ps1 = pspool.tile([128, 256], mybir.dt.float32)

            for c in range(NCHUNK):
                g = gpool.tile([128, NBLK, D], mybir.dt.bfloat16, tag="g")
                # Multi-wait legalization (>=2 sync waits on the extended DMA
                # struct) is handled by Bacc.generate_event_semaphores.
                nc.gpsimd.dma_gather(
                    g[:],
                    wt[:, :],
                    idx_sb[:, c * NBLK * 8 : (c + 1) * NBLK * 8],
                    num_idxs=NBLK * 128,
                    num_idxs_reg=NBLK * 128,
                    elem_size=D,
                    queue_num=c % 4,
                )
                for j in range(NBLK):
                    first = c == 0 and j == 0
                    last = c == NCHUNK - 1 and j == NBLK - 1
                    nc.tensor.matmul(
                        ps0[:], ident_sb[:], g[:, j, 0:512], start=first, stop=last
                    )
                    nc.tensor.matmul(
                        ps1[:], ident_sb[:], g[:, j, 512:768], start=first, stop=last
                    )

            out_sb = opool.tile([128, D], mybir.dt.float32)
            nc.vector.tensor_add(out_sb[:, 0:512], ps0[:], bias_sb[:, 0:512])
            nc.vector.tensor_add(out_sb[:, 512:768], ps1[:], bias_sb[:, 512:768])

            sum_sb = opool.tile([128, 1], mybir.dt.float32)
            nc.vector.reduce_sum(sum_sb[:], out_sb[:], axis=mybir.AxisListType.X)
            msk_sb = opool.tile([128, 1], mybir.dt.float32)
            nc.vector.tensor_scalar(
                msk_sb[:], sum_sb[:], 0.0, None, op0=mybir.AluOpType.is_equal
            )

            nc.sync.dma_start(proj[:], out_sb[:])
            nc.sync.dma_start(mask[:], msk_sb[:])

    nc.finalize()  # Bacc.compile(): wait-splitting, register alloc, DCE
    return nc


def _get_module():
    if "nc" not in _CACHE:
        _CACHE["nc"] = _build_module()
    return _CACHE["nc"]


def _wrap_idxs(tok_core: np.ndarray) -> np.ndarray:
    """token ids [DPC, L] int -> dma_gather idx layout [128, L*8] int16.

    Gather order is l-major (i = l*128 + doc) so gathered block l holds token
    l of all 128 docs, one doc per partition. dma_gather reads idx i from
    partition i%16, column i//16 (replicated across the 8 groups of 16
    partitions).
    """
    flat = np.ascontiguousarray(tok_core.T).reshape(-1)  # [L*DPC], l-major
    wrapped = flat.reshape(L * DPC // 16, 16).T.astype(np.int16)  # [16, L*8]
    return np.tile(wrapped, (8, 1))  # [128, L*8]


def kernel(token_ids, W, b, run_opts=None):
    from concourse import bass_utils

    token_ids = np.asarray(token_ids)
    W = np.asarray(W)
    b = np.asarray(b)

    nc = _get_module()

    wt_bf16 = np.ascontiguousarray(W.T).astype(ml_dtypes.bfloat16)  # [V, D]
    ident = np.eye(128, dtype=ml_dtypes.bfloat16)
    bias_full = np.ascontiguousarray(
        np.broadcast_to(b.astype(np.float32), (128, D))
    )

    in_maps = []
    for core in range(NCORES):
        toks = token_ids[core * DPC : (core + 1) * DPC, :]
        in_maps.append(
            {
                "wt": wt_bf16,
                "idx": _wrap_idxs(toks),
                "ident": ident,
                "bias": bias_full,
            }
        )

    opts = dict(run_opts or {})
    res = bass_utils.run_bass_kernel_spmd(
        nc, in_maps, core_ids=list(range(NCORES)), **opts
    )
    _CACHE["last_results"] = res

    proj = np.concatenate([r["proj"] for r in res.results], axis=0)  # [B, D]
    mask = np.concatenate([r["mask"] for r in res.results], axis=0)  # [B, 1]
    return proj.reshape(B, 1, D), mask.astype(bool)


# revision 17
# speedup vs baseline: 1.0836x; 1.0836x over previous
"""CountVectorizer (histogram + linear projection) Trainium2 kernel.

Math: proj[b, :] = sum_l W[:, token_ids[b, l]] + bias  (counts @ W.T + b),
i.e. an embedding gather-sum over tokens -- the [B, V] histogram is never
materialized.

Sharding: data-parallel over documents, 128 docs per core x 8 cores.
W is replicated to every core as a pre-transposed bf16 table WT [V, D].

Per-core device program:
  - dma_gather (SWDGE indirect DMA) fetches one WT row (1536 B) per token,
    l-major, so gathered block l = [128 docs, 768] for token position l.
  - PE accumulates all 200 blocks into PSUM fp32 via identity-lhsT matmuls
    (out[b, :] += gathered[b, :]).
  - DVE adds bias while copying PSUM->SBUF, then reduce_sum + is_equal
    produce the padding mask.
"""

import numpy as np
import ml_dtypes

B, L, V, D = 1024, 200, 32000, 768
NCORES = 8
DPC = B // NCORES  # 128 docs per core
NBLK = 8           # gathered 128-row blocks per dma_gather call (1024 idxs;
                   # >=1536 idxs/call crashes the SWDGE ucode on this runtime)
NCHUNK = L // NBLK

_CACHE = {}


def _build_module():
    import concourse.mybir as mybir
    import concourse.tile as tile
    from concourse import bacc

    # 4 SWDGE queues: descriptor generation for consecutive gather chunks runs
    # on different Q7 cores concurrently (~8.6us/1024-idx chunk each, but 4 in
    # flight) instead of serializing on one queue.
    nc = bacc.Bacc(
        "TRN2", debug=False, num_swdge_queues=4, dynamic_dma_scratch_size=65536
    )

    wt = nc.dram_tensor("wt", [V, D], mybir.dt.bfloat16, kind="ExternalInput")
    idx = nc.dram_tensor("idx", [128, L * 8], mybir.dt.int16, kind="ExternalInput")
    ident = nc.dram_tensor("ident", [128, 128], mybir.dt.bfloat16, kind="ExternalInput")
    bias = nc.dram_tensor("bias", [128, D], mybir.dt.float32, kind="ExternalInput")
    proj = nc.dram_tensor("proj", [DPC, D], mybir.dt.float32, kind="ExternalOutput")
    mask = nc.dram_tensor("mask", [DPC, 1], mybir.dt.float32, kind="ExternalOutput")

    with tile.TileContext(nc) as tc:
        with (
            tc.tile_pool(name="const", bufs=1) as cpool,
            tc.tile_pool(name="gath", bufs=10) as gpool,
            tc.tile_pool(name="outp", bufs=1) as opool,
            tc.tile_pool(name="ps", bufs=1, space="PSUM") as pspool,
        ):
            # Load the GPSIMD ucode library (dma_gather lives in `mlp`) as the
            # first Pool op so the ~9us library DMA overlaps the input DMAs
            # instead of serializing right before the first gather.
            from concourse import library_config

            nc.gpsimd.load_library(library_config.mlp)

            ident_sb = cpool.tile([128, 128], mybir.dt.bfloat16)
            nc.sync.dma_start(ident_sb[:], ident[:])
            idx_sb = cpool.tile([128, L * 8], mybir.dt.int16)
            nc.sync.dma_start(idx_sb[:], idx[:])
            bias_sb = cpool.tile([128, D], mybir.dt.float32)
            nc.sync.dma_start(bias_sb[:], bias[:])

            ps0 = pspool.tile([128, 512], mybir.dt.float32)
            ps1 = pspool.tile([128, 256], mybir.dt.float32)

            for c in range(NCHUNK):
                g = gpool.tile([128, NBLK, D], mybir.dt.bfloat16, tag="g")
                # Multi-wait legalization (>=2 sync waits on the extended DMA
                # struct) is handled by Bacc.generate_event_semaphores.
                nc.gpsimd.dma_gather(
                    g[:],
                    wt[:, :],
                    idx_sb[:, c * NBLK * 8 : (c + 1) * NBLK * 8],
                    num_idxs=NBLK * 128,
                    num_idxs_reg=NBLK * 128,
                    elem_size=D,
                    queue_num=c % 4,
                )
                for j in range(NBLK):
                    first = c == 0 and j == 0
                    last = c == NCHUNK - 1 and j == NBLK - 1
                    nc.tensor.matmul(
                        ps0[:], ident_sb[:], g[:, j, 0:512], start=first, stop=last
                    )
                    nc.tensor.matmul(
                        ps1[:], ident_sb[:], g[:, j, 512:768], start=first, stop=last
                    )

            out_sb = opool.tile([128, D], mybir.dt.float32)
            nc.vector.tensor_add(out_sb[:, 0:512], ps0[:], bias_sb[:, 0:512])
            nc.vector.tensor_add(out_sb[:, 512:768], ps1[:], bias_sb[:, 512:768])

            sum_sb = opool.tile([128, 1], mybir.dt.float32)
            nc.vector.reduce_sum(sum_sb[:], out_sb[:], axis=mybir.AxisListType.X)
            msk_sb = opool.tile([128, 1], mybir.dt.float32)
            nc.vector.tensor_scalar(
                msk_sb[:], sum_sb[:], 0.0, None, op0=mybir.AluOpType.is_equal
            )

            nc.sync.dma_start(proj[:], out_sb[:])
            nc.sync.dma_start(mask[:], msk_sb[:])

    nc.finalize()  # Bacc.compile(): wait-splitting, register alloc, DCE
    return nc


def _get_module():
    if "nc" not in _CACHE:
        _CACHE["nc"] = _build_module()
    return _CACHE["nc"]


def _wrap_idxs(tok_core: np.ndarray) -> np.ndarray:
    """token ids [DPC, L] int -> dma_gather idx layout [128, L*8] int16.

    Gather order is l-major (i = l*128 + doc) so gathered block l holds token
    l of all 128 docs, one doc per partition. dma_gather reads idx i from
    partition i%16, column i//16 (replicated across the 8 groups of 16
    partitions).
    """
    flat = np.ascontiguousarray(tok_core.T).reshape(-1)  # [L*DPC], l-major
    wrapped = flat.reshape(L * DPC // 16, 16).T.astype(np.int16)  # [16, L*8]
    return np.tile(wrapped, (8, 1))  # [128, L*8]


def kernel(token_ids, W, b, run_opts=None):
    from concourse import bass_utils

    token_ids = np.asarray(token_ids)
    W = np.asarray(W)
    b = np.asarray(b)

    nc = _get_module()

    wt_bf16 = np.ascontiguousarray(W.T).astype(ml_dtypes.bfloat16)  # [V, D]
    ident = np.eye(128, dtype=ml_dtypes.bfloat16)
    bias_full = np.ascontiguousarray(
        np.broadcast_to(b.astype(np.float32), (128, D))
    )

    in_maps = []
    for core in range(NCORES):
        toks = token_ids[core * DPC : (core + 1) * DPC, :]
        in_maps.append(
            {
                "wt": wt_bf16,
                "idx": _wrap_idxs(toks),
                "ident": ident,
                "bias": bias_full,
            }
        )

    opts = dict(run_opts or {})
    res = bass_utils.run_bass_kernel_spmd(
        nc, in_maps, core_ids=list(range(NCORES)), **opts
    )
    _CACHE["last_results"] = res

    proj = np.concatenate([r["proj"] for r in res.results], axis=0)  # [B, D]
    mask = np.concatenate([r["mask"] for r in res.results], axis=0)  # [B, 1]
    return proj.reshape(B, 1, D), mask.astype(bool)
